# revision 1
# baseline (speedup 1.0000x reference)
"""Dev kernel.py — imports gnn_build (will be inlined for submission)."""

import numpy as np

import gnn_build as G

_cache = {}


def _get(x, edge_index, edge_attr, weights):
    cfg = G.Cfg(N=x.shape[0], E=edge_index.shape[1], M=8, KV_SPLIT=32768)
    in_maps, T0, T1 = G.prep(x, edge_index, edge_attr, weights, cfg)
    key = (x.shape, edge_index.shape, T0, T1)
    if key not in _cache:
        _cache[key] = G.build(cfg, T0, T1)
    return _cache[key], in_maps, cfg


def _weights(kw):
    names = [
        "Wq", "bq", "Wk", "bk", "Wv", "bv", "We", "Ws", "bs",
        "Wsp", "bsp", "Wg", "bg", "g1", "b1", "g2", "b2",
    ]
    return tuple(np.asarray(kw[n]) for n in names)


def kernel(x, edge_index, edge_attr, **kw):
    x = np.asarray(x, np.float32)
    edge_index = np.asarray(edge_index)
    edge_attr = np.asarray(edge_attr, np.float32)
    nc, in_maps, cfg = _get(x, edge_index, edge_attr, _weights(kw))
    from concourse.bass_utils import run_bass_kernel_spmd

    res = run_bass_kernel_spmd(nc, in_maps, core_ids=list(range(cfg.M)))
    return G.assemble(res.results, cfg)


def time_kernel(x, edge_index, edge_attr, n_iter=20, **kw):
    """Median per-execution wall time (ns) with device-resident inputs."""
    import jax
    import jax.numpy as jnp
    from jax.sharding import Mesh, PartitionSpec
    from jax.experimental.shard_map import shard_map
    import time as _time

    import concourse.mybir as mybir
    from concourse import bass2jax

    x = np.asarray(x, np.float32)
    edge_index = np.asarray(edge_index)
    edge_attr = np.asarray(edge_attr, np.float32)
    nc, in_maps, cfg = _get(x, edge_index, edge_attr, _weights(kw))
    n_cores = cfg.M

    bass2jax.install_neuronx_cc_hook()
    partition_name = (
        nc.partition_id_tensor.name if nc.partition_id_tensor else None
    )
    in_names, out_names, out_avals, zero_outs = [], [], [], []
    for alloc in nc.m.functions[0].allocations:
        if not isinstance(alloc, mybir.MemoryLocationSet):
            continue
        name = alloc.memorylocations[0].name
        if alloc.kind == "ExternalInput":
            if name != partition_name:
                in_names.append(name)
        elif alloc.kind == "ExternalOutput":
            dt = mybir.dt.np(alloc.dtype)
            out_names.append(name)
            out_avals.append(
                jax.core.ShapedArray(tuple(alloc.tensor_shape), dt)
            )
            zero_outs.append(np.zeros(tuple(alloc.tensor_shape), dt))
    n_params = len(in_names)
    all_in_names = in_names + out_names
    if partition_name is not None:
        all_in_names.append(partition_name)

    def _body(*args):
        operands = list(args)
        if partition_name is not None:
            operands.append(bass2jax.partition_id_tensor())
        outs = bass2jax._bass_exec_p.bind(
            *operands,
            out_avals=tuple(out_avals),
            in_names=tuple(all_in_names),
            out_names=tuple(out_names),
            lowering_input_output_aliases=(),
            sim_require_finite=True,
            sim_require_nnan=True,
            nc=nc,
        )
        return tuple(outs)

    devices = jax.devices()[:n_cores]
    mesh = Mesh(np.asarray(devices), ("core",))
    n_outs = len(out_names)
    sharded = jax.jit(
        shard_map(
            _body,
            mesh=mesh,
            in_specs=(PartitionSpec("core"),) * (n_params + n_outs),
            out_specs=(PartitionSpec("core"),) * n_outs,
            check_rep=False,
        ),
        keep_unused=True,
    )
    concat_in = [
        np.concatenate([np.asarray(in_maps[c][nm]) for c in range(n_cores)], axis=0)
        for nm in in_names
    ]
    concat_zeros = [
        np.zeros((n_cores * z.shape[0], *z.shape[1:]), z.dtype) for z in zero_outs
    ]
    from jax.sharding import NamedSharding

    sh = NamedSharding(mesh, PartitionSpec("core"))
    dev_in = [jax.device_put(a, sh) for a in concat_in]
    dev_zero = [jax.device_put(a, sh) for a in concat_zeros]

    # warmup
    for _ in range(3):
        out = sharded(*dev_in, *dev_zero)
        jax.block_until_ready(out)
    times = []
    for _ in range(n_iter):
        t0 = _time.perf_counter()
        out = sharded(*dev_in, *dev_zero)
        jax.block_until_ready(out)
        times.append((_time.perf_counter() - t0) * 1e9)
    times.sort()
    print(
        f"timing: min {times[0]:.0f} med {times[len(times)//2]:.0f} "
        f"max {times[-1]:.0f} ns over {n_iter} iters"
    )
    return times[len(times) // 2]
